# revision 48
# baseline (speedup 1.0000x reference)
"""GQA attention forward (B=1, T=2048, DIM=2048, H=16, KV=4, HD=128) on 8 trn2 cores.

Sharding: tensor-parallel over heads. Core c owns q-heads {2c, 2c+1} and kv-head
c//2 (kv work duplicated across the pair of cores sharing it).

- bf16 matmul pipeline (PSUM accumulate stays f32); fused quarter-pipeline
  (proj+rope -> attention -> wo per 512-row t-quarter, no phase barriers).
- Host packs x / wq / wk / wv into the exact SBUF layout so every DMA moves
  4-16KB contiguous runs per partition; loads split across both HWDGE queues
  in need-order (x quarter 0 in 4 pieces across both queues).
- rotate_half partition swap via a roll-by-64 permutation matmul on PE
  (no DMA on the rope critical path); rot-matmuls lag one proj so PE never
  waits on the ACT psum->sbuf copy.
- h0 attention scores emitted between the q1 and v projections so the
  serial ACT exp chain (the per-quarter latency floor) overlaps the
  projection phase; first two h1 score tiles emitted before AV(h0) for the
  same reason.
- exp on ACT with 1/sqrt(hd) folded into the activation scale; diagonal
  score matmuls narrowed to the causal region; causal mask on diagonal
  sub-blocks alternates gpsimd affine_select / DVE mask-multiply (two
  parallel chains); softmax denominators via DVE pair-sums of exp tiles +
  PSUM-accumulated ones-matmuls over the pairs (half the dn cols of a
  per-block ones-matmul, no serial DVE chain); last two blocks feed the
  denominator directly so no DVE add gates the tail.
- wo-stage PSUM->SBUF casts alternate ACT/DVE into a [128, 2048] staging
  tile; one 4KB-descriptor store per 128-row block, queues alternating
  (final quarter's stores split across both queues to halve the drain).
Host: pre-packs x/weights into bf16, sums the 8 partial [T, DIM] outputs.
"""

import sys

if "/opt/trn_rl_repo" not in sys.path:
    sys.path.insert(0, "/opt/trn_rl_repo")

import numpy as np

T = 2048
DIM = 2048
H = 16
KV = 4
HD = 128
NCORES = 8
HPC = H // NCORES            # q heads per core = 2
SCALE = float(HD) ** -0.5
ND = DIM // 128              # dim chunks = 16
NT = T // 128                # t blocks = 16
NQC = T // 512               # t quarters = 4

_CACHE = {}


def _build_nc():
    from contextlib import ExitStack

    from concourse import bacc
    import concourse.mybir as mybir
    import concourse.tile as tile
    from concourse.masks import make_identity

    f32 = mybir.dt.float32
    bf16 = mybir.dt.bfloat16
    Exp = mybir.ActivationFunctionType.Exp

    nc = bacc.Bacc("TRN2", target_bir_lowering=False, debug=False,
                   enable_asserts=False)

    # host-packed layouts (see _shard_inputs): per-partition contiguous runs
    xP = nc.dram_tensor("xP", [NQC * 128, ND * 512], bf16,
                        kind="ExternalInput").ap()
    wqP = nc.dram_tensor("wqP", [128, ND * HPC * HD], bf16,
                         kind="ExternalInput").ap()
    wkP = nc.dram_tensor("wkP", [128, ND * HD], bf16,
                         kind="ExternalInput").ap()
    wvP = nc.dram_tensor("wvP", [128, ND * HD], bf16,
                         kind="ExternalInput").ap()
    woT = nc.dram_tensor("woT", [HPC * HD, DIM], bf16,
                         kind="ExternalInput").ap()
    cosT = nc.dram_tensor("cosT", [HD, T], bf16, kind="ExternalInput").ap()
    sinT = nc.dram_tensor("sinT", [HD, T], bf16, kind="ExternalInput").ap()
    out = nc.dram_tensor("out", [T, DIM], bf16, kind="ExternalOutput").ap()

    with tile.TileContext(nc) as tc, ExitStack() as ctx:
        const = ctx.enter_context(tc.tile_pool(name="const", bufs=1))
        wpool = ctx.enter_context(tc.tile_pool(name="wts", bufs=1))
        qkv = ctx.enter_context(tc.tile_pool(name="qkv", bufs=1))

        ident = const.tile([128, 128], bf16)
        make_identity(nc, ident)
        ones_s = const.tile([128, 128], bf16)
        nc.vector.memset(ones_s, 1.0)
        # partition-roll-by-64 permutation (for rotate_half via PE matmul)
        perm = const.tile([128, 128], bf16)
        nc.scalar.dma_start(perm[0:64, :], ident[64:128, :])
        nc.scalar.dma_start(perm[64:128, :], ident[0:64, :])

        qT_s = qkv.tile([128, HPC * T], bf16)
        kT_s = qkv.tile([128, T], bf16)
        v_s = qkv.tile([128, NT * HD], bf16)   # natural [t%128, hd] per t-block
        aT_s = [qkv.tile([128, T], bf16, name=f"aT{h}") for h in range(HPC)]
        vT_stage = qkv.tile([128, T], bf16)

        # resident x: one [128, 16*512] tile per t-quarter
        x_s = [qkv.tile([128, ND * 512], bf16, name=f"x{q}")
               for q in range(NQC)]
        wq_s = wpool.tile([128, ND, HPC * HD], bf16)
        wk_s = wpool.tile([128, ND, HD], bf16)
        wv_s = wpool.tile([128, ND, HD], bf16)
        wo_s = wpool.tile([128, HPC, DIM], bf16)
        cos_s = const.tile([128, T], bf16)
        sin_s = const.tile([128, T], bf16)

        QX = ND * 512 // 4  # quarter of an x quarter, in free elems

        # DMA issue order = need order, split across the two HWDGE queues
        # (the scalar queue spins up several us late, so everything needed
        # in the first ~10us rides the sync queue).
        # sync  (q1):  wk, x0 pieces 0-2, wq halves, wv, x1 (x2/x3 in-loop)
        # scalar(q10): x0 piece 3, cos, sin, wo
        nc.sync.dma_start(wk_s, wkP.rearrange("p (d n) -> p d n", d=ND))
        nc.sync.dma_start(x_s[0][:, 0:QX], xP[0:128, 0:QX])
        nc.scalar.dma_start(x_s[0][:, 2 * QX:3 * QX], xP[0:128, 2 * QX:3 * QX])
        nc.scalar.dma_start(x_s[0][:, 3 * QX:], xP[0:128, 3 * QX:])
        nc.sync.dma_start(x_s[0][:, QX:2 * QX], xP[0:128, QX:2 * QX])
        nc.sync.dma_start(
            wq_s[:, 0:ND // 2, :],
            wqP[:, 0:ND * HPC * HD // 2].rearrange(
                "p (d n) -> p d n", d=ND // 2))
        nc.sync.dma_start(
            wq_s[:, ND // 2:, :],
            wqP[:, ND * HPC * HD // 2:].rearrange(
                "p (d n) -> p d n", d=ND // 2))
        nc.scalar.dma_start(cos_s, cosT)
        nc.scalar.dma_start(sin_s, sinT)
        nc.scalar.dma_start(wv_s, wvP.rearrange("p (d n) -> p d n", d=ND))
        nc.scalar.dma_start(wo_s, woT.rearrange("(h p) n -> p h n", p=128))
        nc.sync.dma_start(x_s[1], xP[128:256, :])

        rp = ctx.enter_context(tc.tile_pool(name="rope", bufs=4))
        pps = ctx.enter_context(tc.tile_pool(name="pps", bufs=2, space="PSUM"))
        sps = ctx.enter_context(tc.tile_pool(name="sps", bufs=2, space="PSUM"))
        warm_src = const.tile([128, 512], bf16)
        nc.vector.memset(warm_src, 0.0)
        # warmup cycles through the score-psum buffers so they start zeroed
        for w in range(18):
            warm_ps = sps.tile([128, 512], f32, tag="s", name=f"warm{w}")
            nc.tensor.matmul(warm_ps, ones_s, warm_src,
                             start=True, stop=True)
        # causal masks for the 4 diagonal sub-block offsets (built on gpsimd,
        # after warmup emission, so nothing on the PE path waits for them):
        # mask_j[k, q] = 1 iff q >= k + j*128
        masks = []
        for j in range(4):
            mj = const.tile([128, 512], bf16, name=f"mask{j}")
            nc.gpsimd.memset(mj, 1.0)
            nc.gpsimd.affine_select(
                out=mj, in_=mj, compare_op=mybir.AluOpType.is_ge,
                fill=0.0, base=-j * 128, channel_multiplier=-1,
                pattern=[[1, 512]])
            masks.append(mj)
        otp = ctx.enter_context(tc.tile_pool(name="otp", bufs=1, space="PSUM"))
        dnp = ctx.enter_context(tc.tile_pool(name="dnp", bufs=1, space="PSUM"))
        wops = ctx.enter_context(tc.tile_pool(name="wops", bufs=2, space="PSUM"))
        ppool = ctx.enter_context(tc.tile_pool(name="pp", bufs=20))
        rcp = ctx.enter_context(tc.tile_pool(name="rcp", bufs=2))
        prs = ctx.enter_context(tc.tile_pool(name="prs", bufs=6))
        ostage = ctx.enter_context(tc.tile_pool(name="ost", bufs=3))

        # pre-zero the exp-tile pool buffers during the startup DMA wait:
        # narrowed diagonal exps leave [0:lo] untouched, and uninitialized
        # SBUF could hold NaN patterns that survive a mask multiply
        for w in range(20):
            pz = ppool.tile([128, 512], bf16, tag="p", name=f"pz{w}")
            nc.vector.memset(pz, 0.0)

        def rope_rot(u, c0):
            # PE: permuted copy (partition roll by 64) into PSUM
            us = u[:, c0:c0 + 512]
            rot_ps = sps.tile([128, 512], f32, tag="s", name="rotp")
            nc.tensor.matmul(rot_ps, perm, us, start=True, stop=True)
            return rot_ps

        def rope_fin(u, c0, t0, rot_ps):
            # DVE: u = u*cos + rot*sin  (sign folded into sin table)
            us = u[:, c0:c0 + 512]
            tmp = rp.tile([128, 512], bf16, tag="rtmp")
            nc.vector.tensor_mul(tmp, us, cos_s[:, t0:t0 + 512])
            rot = rp.tile([128, 512], bf16, tag="rot")
            nc.vector.tensor_mul(rot, rot_ps, sin_s[:, t0:t0 + 512])
            nc.vector.tensor_add(us, tmp, rot)

        def proj(acc_tag, w_ap, xt, dst, c0):
            acc = pps.tile([128, 512], f32, tag="pps", name=acc_tag)
            for d in range(ND):
                nc.tensor.matmul(acc, w_ap(d), xt[:, d * 512:(d + 1) * 512],
                                 start=(d == 0), stop=(d == ND - 1))
            nc.scalar.copy(dst[:, c0:c0 + 512], acc)

        def v_transposes(tq):
            # psum->sbuf copies ride DVE: ACT is busy with the h0 exps here
            for tb in range(tq * 4, tq * 4 + 4):
                vt = pps.tile([128, 128], bf16, tag="pps", name=f"vt{tb}")
                nc.tensor.transpose(
                    vt, vT_stage[:, tb * 128:(tb + 1) * 128], ident)
                nc.vector.tensor_copy(v_s[:, tb * HD:(tb + 1) * HD], vt)

        def attn_scores(h, qc, kb_lo=0, kb_hi=None, ptiles=None, pairs=None):
            qTh = qT_s[:, h * T + qc * 512:h * T + (qc + 1) * 512]
            nkb = 4 * qc + 4
            if kb_hi is None:
                kb_hi = nkb
            ptiles = [] if ptiles is None else ptiles
            # pair-sum all k-blocks except the last two: the final two feed
            # the denominator directly so the tail isn't gated by a DVE add
            npr_kb = nkb - 2
            pairs = [] if pairs is None else pairs
            for idx in range(kb_lo, kb_hi):
                kb = idx
                s_ps = sps.tile([128, 512], f32, tag="s", name=f"s{h}_{qc}_{kb}")
                j = kb - 4 * qc
                # diagonal sub-blocks only need the upper-left part; stale
                # psum columns turn into garbage exp values that the mask
                # zeroes (sps buffers are warmup-zeroed, so always finite)
                lo = j * 128 if j >= 1 else 0
                nc.tensor.matmul(
                    s_ps[:, lo:], kT_s[:, kb * 128:(kb + 1) * 128],
                    qTh[:, lo:], start=True, stop=True)
                p_sb = ppool.tile([128, 512], bf16, tag="p",
                                  name=f"p{h}_{qc}_{kb}")
                # exp only the causal region of diagonal blocks: the mask
                # (full-width select fill / mul-by-0) covers the stale rest,
                # and the ACT exp chain is the attention-phase serial floor
                nc.scalar.activation(p_sb[:, lo:], s_ps[:, lo:], Exp,
                                     scale=SCALE)
                if j >= 0:
                    # causal mask on the diagonal sub-blocks; alternate
                    # engines so the 4 masks form two parallel chains
                    if j % 2 == 0:
                        nc.gpsimd.affine_select(
                            out=p_sb, in_=p_sb,
                            compare_op=mybir.AluOpType.is_ge,
                            fill=0.0, base=qc * 512 - kb * 128,
                            channel_multiplier=-1, pattern=[[1, 512]])
                    else:
                        nc.vector.tensor_mul(p_sb, p_sb, masks[j])
                ptiles.append(p_sb)
                if idx % 2 == 1 and idx < npr_kb:
                    # independent pair-sums (of loop-adjacent tiles) feed the
                    # denominator matmuls; late non-diagonal pairs ride
                    # gpsimd to relieve the congested quarter-end DVE queue
                    sp = prs.tile([128, 512], bf16, tag="pr",
                                  name=f"pr{h}_{qc}_{idx // 2}")
                    if kb < 4 * qc and kb >= npr_kb - 6:
                        nc.gpsimd.tensor_add(sp, ptiles[idx - 1], p_sb)
                    else:
                        nc.vector.tensor_add(sp, ptiles[idx - 1], p_sb)
                    pairs.append(sp)
            return ptiles, pairs

        def attn_dn(h, qc, ptiles, pairs, upto=None):
            # denominator matmuls (moving operands are the DVE pair-sums and
            # the last two masked exp tiles)
            nkb = 4 * qc + 4
            dn_moving = pairs + ptiles[nkb - 2:]
            dn = dnp.tile([128, 512], f32, tag="dn", name=f"dn{h}_{qc}")
            n = len(dn_moving) if upto is None else upto
            for i in range(n):
                nc.tensor.matmul(dn, ones_s, dn_moving[i],
                                 start=(i == 0), stop=(i == len(dn_moving) - 1))
            return dn, dn_moving

        def attn_av(h, qc, ptiles, dn, dn_moving, ndn):
            nkb = 4 * qc + 4
            oT = otp.tile([128, 512], f32, tag="oT", name=f"oT{h}_{qc}")
            npr = len(dn_moving)
            for idx in range(nkb):
                kb = idx
                # diagonal blocks only contribute to the causal region; the
                # narrowed accumulate is safe because kb==0 (start) is always
                # full-width and stop is a hardware no-op
                j = kb - 4 * qc
                lo = j * 128 if j >= 1 else 0
                nc.tensor.matmul(
                    oT[:, lo:], v_s[:, kb * HD:(kb + 1) * HD],
                    ptiles[idx][:, lo:],
                    start=(idx == 0), stop=(idx == nkb - 1))
                # interleave remaining denominator matmuls a few slots behind
                # so their DVE pair-sums are ready when PE reaches them
                while ndn < npr - 2 and 2 * ndn + 3 <= idx:
                    nc.tensor.matmul(dn, ones_s, dn_moving[ndn],
                                     start=(ndn == 0), stop=(ndn == npr - 1))
                    ndn += 1
            while ndn < npr:
                nc.tensor.matmul(dn, ones_s, dn_moving[ndn],
                                 start=(ndn == 0), stop=(ndn == npr - 1))
                ndn += 1
            # reciprocal + normalize in halves so the first wo matmuls of the
            # quarter can start before the full 512-wide normalize finishes
            rec = rcp.tile([128, 512], f32, tag="rec")
            aTh = aT_s[h][:, qc * 512:(qc + 1) * 512]
            for c in (0, 256):
                nc.vector.reciprocal_approx_fast(
                    rec[:, c:c + 256], dn[:, c:c + 256])
                nc.vector.tensor_mul(
                    aTh[:, c:c + 256], oT[:, c:c + 256], rec[:, c:c + 256])

        def wo_block(qc):
            for tb in range(qc * 4, qc * 4 + 4):
                ob = ostage.tile([128, DIM], bf16, tag="ob")
                for n4 in range(4):
                    op = wops.tile([128, 512], f32, tag="op")
                    for h in range(HPC):
                        nc.tensor.matmul(
                            op, aT_s[h][:, tb * 128:(tb + 1) * 128],
                            wo_s[:, h, n4 * 512:(n4 + 1) * 512],
                            start=(h == 0), stop=(h == HPC - 1))
                    if n4 % 2 == 0:
                        nc.scalar.copy(ob[:, n4 * 512:(n4 + 1) * 512], op)
                    else:
                        nc.vector.tensor_copy(
                            ob[:, n4 * 512:(n4 + 1) * 512], op)
                if qc == NQC - 1:
                    # final quarter: halve each store across both queues so
                    # the last store drains in half the time
                    rows = out[tb * 128:(tb + 1) * 128, :]
                    nc.sync.dma_start(rows[:, 0:DIM // 2], ob[:, 0:DIM // 2])
                    nc.scalar.dma_start(rows[:, DIM // 2:], ob[:, DIM // 2:])
                else:
                    eng = nc.sync if tb % 2 == 0 else nc.scalar
                    eng.dma_start(out[tb * 128:(tb + 1) * 128, :], ob)

        for tq in range(NQC):
            if tq == 0:
                nc.sync.dma_start(x_s[2], xP[256:384, :])
            elif tq == 1:
                nc.sync.dma_start(x_s[3], xP[384:512, :])
            t0 = tq * 512
            xt = x_s[tq]
            # proj + rope, interleaved so PE never waits on ACT copies;
            # h0 scores run between q1-proj and v-proj so the serial ACT
            # exp chain starts ~7us earlier and overlaps the projections
            proj(f"k{tq}", lambda d: wk_s[:, d, :], xt, kT_s, t0)
            proj(f"q0_{tq}", lambda d: wq_s[:, d, 0:HD], xt, qT_s, t0)
            rk = rope_rot(kT_s, t0)
            proj(f"q1_{tq}", lambda d: wq_s[:, d, HD:2 * HD],
                 xt, qT_s, T + t0)
            rope_fin(kT_s, t0, t0, rk)
            rq0 = rope_rot(qT_s, t0)
            rope_fin(qT_s, t0, t0, rq0)
            pt0, pr0 = attn_scores(0, tq)
            proj(f"v{tq}", lambda d: wv_s[:, d, :], xt, vT_stage, t0)
            rq1 = rope_rot(qT_s, T + t0)
            rope_fin(qT_s, T + t0, t0, rq1)
            # early denominator matmuls (ready pairs) cover the v-copy wait
            n0 = max(0, len(pr0) - 4)
            dn0, mv0 = attn_dn(0, tq, pt0, pr0, upto=n0)
            v_transposes(tq)
            # first two h1 score tiles before AV h0: their ACT exps queue
            # right behind the h0 exps instead of idling ACT
            pt1, pr1 = attn_scores(1, tq, kb_lo=0, kb_hi=2)
            attn_av(0, tq, pt0, dn0, mv0, n0)
            attn_scores(1, tq, kb_lo=2, ptiles=pt1, pairs=pr1)
            dn1, mv1 = attn_dn(1, tq, pt1, pr1, upto=0)
            attn_av(1, tq, pt1, dn1, mv1, 0)
            wo_block(tq)

    nc.compile()
    return nc


def _shard_inputs(x, wq, wk, wv, wo, cos, sin):
    import ml_dtypes

    bf = ml_dtypes.bfloat16

    # x packed per t-quarter into SBUF layout: [4*128, 16*512] where row
    # (q*128 + p), col (d*512 + t) = x[q*512 + t, d*128 + p]
    x2 = x.reshape(T, DIM)
    xPh = np.ascontiguousarray(
        x2.reshape(NQC, 512, ND, 128).transpose(0, 3, 2, 1)
        .reshape(NQC * 128, ND * 512)).astype(bf)

    cosTh = np.ascontiguousarray(cos.T).astype(bf)
    # rotate_half sign fold: out = u*cos + u_rot*sin_signed
    sinTh = np.ascontiguousarray(sin.T).copy()
    sinTh[: HD // 2, :] *= -1.0
    sinTh = sinTh.astype(bf)

    def packw(w_slice):
        # [n, DIM] -> [128 p, 16 d, n] -> [128, 16*n]
        n = w_slice.shape[0]
        return np.ascontiguousarray(
            w_slice.reshape(n, ND, 128).transpose(2, 1, 0)
            .reshape(128, ND * n)).astype(bf)

    in_maps = []
    for c in range(NCORES):
        g = c // 2
        in_maps.append({
            "xP": xPh,
            "wqP": packw(wq[c * HPC * HD:(c + 1) * HPC * HD, :]),
            "wkP": packw(wk[g * HD:(g + 1) * HD, :]),
            "wvP": packw(wv[g * HD:(g + 1) * HD, :]),
            "woT": np.ascontiguousarray(
                wo[:, c * HPC * HD:(c + 1) * HPC * HD].T).astype(bf),
            "cosT": cosTh,
            "sinT": sinTh,
        })
    return in_maps


def _get_exec():
    """Build (once) a cached jitted SPMD executable over the 8 cores.

    Mirrors bass2jax.run_bass_via_pjrt's multi-core branch, but caches the
    jitted callable so repeat kernel() calls don't re-trace/re-lower.
    """
    if "exec" in _CACHE:
        return _CACHE["exec"]

    import jax
    from jax.sharding import Mesh, PartitionSpec
    from jax.experimental.shard_map import shard_map
    from concourse import bass2jax
    import concourse.mybir as mybir

    if "nc" not in _CACHE:
        _CACHE["nc"] = _build_nc()
    nc = _CACHE["nc"]

    bass2jax.install_neuronx_cc_hook()

    part_name = (nc.partition_id_tensor.name
                 if nc.partition_id_tensor else None)
    in_names, out_names, out_avals = [], [], []
    for alloc in nc.m.functions[0].allocations:
        if not isinstance(alloc, mybir.MemoryLocationSet):
            continue
        name = alloc.memorylocations[0].name
        if alloc.kind == "ExternalInput":
            if name != part_name:
                in_names.append(name)
        elif alloc.kind == "ExternalOutput":
            out_names.append(name)
            out_avals.append(jax.core.ShapedArray(
                tuple(alloc.tensor_shape), mybir.dt.np(alloc.dtype)))

    bind_names = in_names + out_names
    if part_name is not None:
        bind_names = bind_names + [part_name]

    def _body(*args):
        operands = list(args)
        if part_name is not None:
            operands.append(bass2jax.partition_id_tensor())
        outs = bass2jax._bass_exec_p.bind(
            *operands,
            out_avals=tuple(out_avals),
            in_names=tuple(bind_names),
            out_names=tuple(out_names),
            lowering_input_output_aliases=(),
            sim_require_finite=True,
            sim_require_nnan=True,
            nc=nc,
        )
        return tuple(outs)

    devices = jax.devices()[:NCORES]
    mesh = Mesh(np.asarray(devices), ("core",))
    n_in = len(in_names)
    n_out = len(out_names)
    sharded = jax.jit(
        shard_map(
            _body, mesh=mesh,
            in_specs=(PartitionSpec("core"),) * (n_in + n_out),
            out_specs=(PartitionSpec("core"),) * n_out,
            check_rep=False,
        ),
        donate_argnums=tuple(range(n_in, n_in + n_out)),
        keep_unused=True,
    )
    _CACHE["body"] = _body
    _CACHE["exec"] = (sharded, in_names, out_names, out_avals, mesh)
    return _CACHE["exec"]


def _concat_inputs(in_maps, in_names):
    return [
        np.concatenate([in_maps[c][name] for c in range(NCORES)], axis=0)
        for name in in_names
    ]


def _zero_outs(out_avals):
    return [
        np.zeros((NCORES * a.shape[0], *a.shape[1:]), a.dtype)
        for a in out_avals
    ]


def kernel(**inputs):
    sharded, in_names, out_names, out_avals, _ = _get_exec()

    in_maps = _shard_inputs(
        np.asarray(inputs["x"], dtype=np.float32),
        np.asarray(inputs["wq"], dtype=np.float32),
        np.asarray(inputs["wk"], dtype=np.float32),
        np.asarray(inputs["wv"], dtype=np.float32),
        np.asarray(inputs["wo"], dtype=np.float32),
        np.asarray(inputs["cos"], dtype=np.float32),
        np.asarray(inputs["sin"], dtype=np.float32),
    )
    concat_in = _concat_inputs(in_maps, in_names)
    out_arrs = sharded(*concat_in, *_zero_outs(out_avals))

    full = np.asarray(out_arrs[out_names.index("out")])
    acc = full.reshape(NCORES, T, DIM).astype(np.float32).sum(axis=0)
    return acc.reshape(1, T, DIM)


# revision 49
# speedup vs baseline: 1.0644x; 1.0644x over previous
"""GQA attention forward (B=1, T=2048, DIM=2048, H=16, KV=4, HD=128) on 8 trn2 cores.

Sharding: tensor-parallel over heads. Core c owns q-heads {2c, 2c+1} and kv-head
c//2 (kv work duplicated across the pair of cores sharing it).

- bf16 matmul pipeline (PSUM accumulate stays f32); fused quarter-pipeline
  (proj+rope -> attention -> wo per 512-row t-quarter, no phase barriers).
- Host packs x / wq / wk / wv into the exact SBUF layout so every DMA moves
  4-16KB contiguous runs per partition; loads split across both HWDGE queues
  in need-order (x quarter 0 in 4 pieces across both queues).
- rotate_half partition swap via a roll-by-64 permutation matmul on PE
  (no DMA on the rope critical path); rot-matmuls lag one proj so PE never
  waits on the ACT psum->sbuf copy.
- h0 attention scores emitted between the q1 and v projections so the
  serial ACT exp chain (the per-quarter latency floor) overlaps the
  projection phase; first two h1 score tiles emitted before AV(h0) for the
  same reason.
- exp on ACT with 1/sqrt(hd) folded into the activation scale; diagonal
  score matmuls narrowed to the causal region; causal mask on diagonal
  sub-blocks alternates gpsimd affine_select / DVE mask-multiply (two
  parallel chains); softmax denominators via DVE pair-sums of exp tiles +
  PSUM-accumulated ones-matmuls over the pairs (half the dn cols of a
  per-block ones-matmul, no serial DVE chain); last two blocks feed the
  denominator directly so no DVE add gates the tail.
- wo-stage PSUM->SBUF casts alternate ACT/DVE into a [128, 2048] staging
  tile; one 4KB-descriptor store per 128-row block, queues alternating
  (final quarter's stores split across both queues to halve the drain).
Host: pre-packs x/weights into bf16, sums the 8 partial [T, DIM] outputs.
"""

import sys

if "/opt/trn_rl_repo" not in sys.path:
    sys.path.insert(0, "/opt/trn_rl_repo")

import numpy as np

T = 2048
DIM = 2048
H = 16
KV = 4
HD = 128
NCORES = 8
HPC = H // NCORES            # q heads per core = 2
SCALE = float(HD) ** -0.5
ND = DIM // 128              # dim chunks = 16
NT = T // 128                # t blocks = 16
NQC = T // 512               # t quarters = 4

_CACHE = {}


def _build_nc():
    from contextlib import ExitStack

    from concourse import bacc
    import concourse.mybir as mybir
    import concourse.tile as tile
    from concourse.masks import make_identity

    f32 = mybir.dt.float32
    bf16 = mybir.dt.bfloat16
    Exp = mybir.ActivationFunctionType.Exp

    nc = bacc.Bacc("TRN2", target_bir_lowering=False, debug=False,
                   enable_asserts=False)

    # host-packed layouts (see _shard_inputs): per-partition contiguous runs
    xP = nc.dram_tensor("xP", [NQC * 128, ND * 512], bf16,
                        kind="ExternalInput").ap()
    wqP = nc.dram_tensor("wqP", [128, ND * HPC * HD], bf16,
                         kind="ExternalInput").ap()
    wkP = nc.dram_tensor("wkP", [128, ND * HD], bf16,
                         kind="ExternalInput").ap()
    wvP = nc.dram_tensor("wvP", [128, ND * HD], bf16,
                         kind="ExternalInput").ap()
    woT = nc.dram_tensor("woT", [HPC * HD, DIM], bf16,
                         kind="ExternalInput").ap()
    cosT = nc.dram_tensor("cosT", [HD, T], bf16, kind="ExternalInput").ap()
    sinT = nc.dram_tensor("sinT", [HD, T], bf16, kind="ExternalInput").ap()
    out = nc.dram_tensor("out", [T, DIM], bf16, kind="ExternalOutput").ap()

    with tile.TileContext(nc) as tc, ExitStack() as ctx:
        const = ctx.enter_context(tc.tile_pool(name="const", bufs=1))
        wpool = ctx.enter_context(tc.tile_pool(name="wts", bufs=1))
        qkv = ctx.enter_context(tc.tile_pool(name="qkv", bufs=1))

        ident = const.tile([128, 128], bf16)
        make_identity(nc, ident)
        ones_s = const.tile([128, 128], bf16)
        nc.vector.memset(ones_s, 1.0)
        # partition-roll-by-64 permutation (for rotate_half via PE matmul)
        perm = const.tile([128, 128], bf16)
        nc.scalar.dma_start(perm[0:64, :], ident[64:128, :])
        nc.scalar.dma_start(perm[64:128, :], ident[0:64, :])

        qT_s = qkv.tile([128, HPC * T], bf16)
        kT_s = qkv.tile([128, T], bf16)
        v_s = qkv.tile([128, NT * HD], bf16)   # natural [t%128, hd] per t-block
        aT_s = [qkv.tile([128, T], bf16, name=f"aT{h}") for h in range(HPC)]
        vT_stage = qkv.tile([128, T], bf16)

        # resident x: one [128, 16*512] tile per t-quarter
        x_s = [qkv.tile([128, ND * 512], bf16, name=f"x{q}")
               for q in range(NQC)]
        wq_s = wpool.tile([128, ND, HPC * HD], bf16)
        wk_s = wpool.tile([128, ND, HD], bf16)
        wv_s = wpool.tile([128, ND, HD], bf16)
        wo_s = wpool.tile([128, HPC, DIM], bf16)
        cos_s = const.tile([128, T], bf16)
        sin_s = const.tile([128, T], bf16)

        QX = ND * 512 // 4  # quarter of an x quarter, in free elems

        # DMA issue order = need order, split across the two HWDGE queues
        # (the scalar queue spins up several us late, so everything needed
        # in the first ~10us rides the sync queue).
        # sync  (q1):  wk, x0 pieces 0-2, wq halves, wv, x1 (x2/x3 in-loop)
        # scalar(q10): x0 piece 3, cos, sin, wo
        nc.sync.dma_start(wk_s, wkP.rearrange("p (d n) -> p d n", d=ND))
        nc.sync.dma_start(x_s[0][:, 0:QX], xP[0:128, 0:QX])
        nc.scalar.dma_start(x_s[0][:, 2 * QX:3 * QX], xP[0:128, 2 * QX:3 * QX])
        nc.scalar.dma_start(x_s[0][:, 3 * QX:], xP[0:128, 3 * QX:])
        nc.sync.dma_start(x_s[0][:, QX:2 * QX], xP[0:128, QX:2 * QX])
        nc.sync.dma_start(
            wq_s[:, 0:ND // 2, :],
            wqP[:, 0:ND * HPC * HD // 2].rearrange(
                "p (d n) -> p d n", d=ND // 2))
        nc.sync.dma_start(
            wq_s[:, ND // 2:, :],
            wqP[:, ND * HPC * HD // 2:].rearrange(
                "p (d n) -> p d n", d=ND // 2))
        nc.scalar.dma_start(cos_s, cosT)
        nc.scalar.dma_start(sin_s, sinT)
        nc.sync.dma_start(wv_s, wvP.rearrange("p (d n) -> p d n", d=ND))
        nc.scalar.dma_start(wo_s, woT.rearrange("(h p) n -> p h n", p=128))
        nc.sync.dma_start(x_s[1], xP[128:256, :])

        rp = ctx.enter_context(tc.tile_pool(name="rope", bufs=4))
        pps = ctx.enter_context(tc.tile_pool(name="pps", bufs=2, space="PSUM"))
        sps = ctx.enter_context(tc.tile_pool(name="sps", bufs=2, space="PSUM"))
        warm_src = const.tile([128, 512], bf16)
        nc.vector.memset(warm_src, 0.0)
        # warmup cycles through the score-psum buffers so they start zeroed
        for w in range(18):
            warm_ps = sps.tile([128, 512], f32, tag="s", name=f"warm{w}")
            nc.tensor.matmul(warm_ps, ones_s, warm_src,
                             start=True, stop=True)
        # causal masks for the 4 diagonal sub-block offsets (built on gpsimd,
        # after warmup emission, so nothing on the PE path waits for them):
        # mask_j[k, q] = 1 iff q >= k + j*128
        masks = []
        for j in range(4):
            mj = const.tile([128, 512], bf16, name=f"mask{j}")
            nc.gpsimd.memset(mj, 1.0)
            nc.gpsimd.affine_select(
                out=mj, in_=mj, compare_op=mybir.AluOpType.is_ge,
                fill=0.0, base=-j * 128, channel_multiplier=-1,
                pattern=[[1, 512]])
            masks.append(mj)
        otp = ctx.enter_context(tc.tile_pool(name="otp", bufs=1, space="PSUM"))
        dnp = ctx.enter_context(tc.tile_pool(name="dnp", bufs=1, space="PSUM"))
        wops = ctx.enter_context(tc.tile_pool(name="wops", bufs=2, space="PSUM"))
        ppool = ctx.enter_context(tc.tile_pool(name="pp", bufs=20))
        rcp = ctx.enter_context(tc.tile_pool(name="rcp", bufs=2))
        prs = ctx.enter_context(tc.tile_pool(name="prs", bufs=6))
        ostage = ctx.enter_context(tc.tile_pool(name="ost", bufs=3))

        # pre-zero the exp-tile pool buffers during the startup DMA wait:
        # narrowed diagonal exps leave [0:lo] untouched, and uninitialized
        # SBUF could hold NaN patterns that survive a mask multiply
        for w in range(20):
            pz = ppool.tile([128, 512], bf16, tag="p", name=f"pz{w}")
            nc.vector.memset(pz, 0.0)

        def rope_rot(u, c0):
            # PE: permuted copy (partition roll by 64) into PSUM
            us = u[:, c0:c0 + 512]
            rot_ps = sps.tile([128, 512], f32, tag="s", name="rotp")
            nc.tensor.matmul(rot_ps, perm, us, start=True, stop=True)
            return rot_ps

        def rope_fin(u, c0, t0, rot_ps):
            # DVE: u = u*cos + rot*sin  (sign folded into sin table)
            us = u[:, c0:c0 + 512]
            tmp = rp.tile([128, 512], bf16, tag="rtmp")
            nc.vector.tensor_mul(tmp, us, cos_s[:, t0:t0 + 512])
            rot = rp.tile([128, 512], bf16, tag="rot")
            nc.vector.tensor_mul(rot, rot_ps, sin_s[:, t0:t0 + 512])
            nc.vector.tensor_add(us, tmp, rot)

        def proj(acc_tag, w_ap, xt, dst, c0):
            acc = pps.tile([128, 512], f32, tag="pps", name=acc_tag)
            for d in range(ND):
                nc.tensor.matmul(acc, w_ap(d), xt[:, d * 512:(d + 1) * 512],
                                 start=(d == 0), stop=(d == ND - 1))
            nc.scalar.copy(dst[:, c0:c0 + 512], acc)

        def v_transposes(tq):
            # psum->sbuf copies ride DVE: ACT is busy with the h0 exps here
            for tb in range(tq * 4, tq * 4 + 4):
                vt = pps.tile([128, 128], bf16, tag="pps", name=f"vt{tb}")
                nc.tensor.transpose(
                    vt, vT_stage[:, tb * 128:(tb + 1) * 128], ident)
                nc.vector.tensor_copy(v_s[:, tb * HD:(tb + 1) * HD], vt)

        def attn_scores(h, qc, kb_lo=0, kb_hi=None, ptiles=None, pairs=None):
            qTh = qT_s[:, h * T + qc * 512:h * T + (qc + 1) * 512]
            nkb = 4 * qc + 4
            if kb_hi is None:
                kb_hi = nkb
            ptiles = [] if ptiles is None else ptiles
            # pair-sum all k-blocks except the last two: the final two feed
            # the denominator directly so the tail isn't gated by a DVE add
            npr_kb = nkb - 2
            pairs = [] if pairs is None else pairs
            for idx in range(kb_lo, kb_hi):
                kb = idx
                s_ps = sps.tile([128, 512], f32, tag="s", name=f"s{h}_{qc}_{kb}")
                j = kb - 4 * qc
                # diagonal sub-blocks only need the upper-left part; stale
                # psum columns turn into garbage exp values that the mask
                # zeroes (sps buffers are warmup-zeroed, so always finite)
                lo = j * 128 if j >= 1 else 0
                nc.tensor.matmul(
                    s_ps[:, lo:], kT_s[:, kb * 128:(kb + 1) * 128],
                    qTh[:, lo:], start=True, stop=True)
                p_sb = ppool.tile([128, 512], bf16, tag="p",
                                  name=f"p{h}_{qc}_{kb}")
                # exp only the causal region of diagonal blocks: the mask
                # (full-width select fill / mul-by-0) covers the stale rest,
                # and the ACT exp chain is the attention-phase serial floor
                nc.scalar.activation(p_sb[:, lo:], s_ps[:, lo:], Exp,
                                     scale=SCALE)
                if j >= 0:
                    # causal mask on the diagonal sub-blocks; alternate
                    # engines so the 4 masks form two parallel chains
                    if j % 2 == 0:
                        nc.gpsimd.affine_select(
                            out=p_sb, in_=p_sb,
                            compare_op=mybir.AluOpType.is_ge,
                            fill=0.0, base=qc * 512 - kb * 128,
                            channel_multiplier=-1, pattern=[[1, 512]])
                    else:
                        nc.vector.tensor_mul(p_sb, p_sb, masks[j])
                ptiles.append(p_sb)
                if idx % 2 == 1 and idx < npr_kb:
                    # independent pair-sums (of loop-adjacent tiles) feed the
                    # denominator matmuls; late non-diagonal pairs ride
                    # gpsimd to relieve the congested quarter-end DVE queue
                    sp = prs.tile([128, 512], bf16, tag="pr",
                                  name=f"pr{h}_{qc}_{idx // 2}")
                    if kb < 4 * qc and kb >= npr_kb - 6:
                        nc.gpsimd.tensor_add(sp, ptiles[idx - 1], p_sb)
                    else:
                        nc.vector.tensor_add(sp, ptiles[idx - 1], p_sb)
                    pairs.append(sp)
            return ptiles, pairs

        def attn_dn(h, qc, ptiles, pairs, upto=None):
            # denominator matmuls (moving operands are the DVE pair-sums and
            # the last two masked exp tiles)
            nkb = 4 * qc + 4
            dn_moving = pairs + ptiles[nkb - 2:]
            dn = dnp.tile([128, 512], f32, tag="dn", name=f"dn{h}_{qc}")
            n = len(dn_moving) if upto is None else upto
            for i in range(n):
                nc.tensor.matmul(dn, ones_s, dn_moving[i],
                                 start=(i == 0), stop=(i == len(dn_moving) - 1))
            return dn, dn_moving

        def attn_av(h, qc, ptiles, dn, dn_moving, ndn):
            nkb = 4 * qc + 4
            oT = otp.tile([128, 512], f32, tag="oT", name=f"oT{h}_{qc}")
            npr = len(dn_moving)
            for idx in range(nkb):
                kb = idx
                # diagonal blocks only contribute to the causal region; the
                # narrowed accumulate is safe because kb==0 (start) is always
                # full-width and stop is a hardware no-op
                j = kb - 4 * qc
                lo = j * 128 if j >= 1 else 0
                nc.tensor.matmul(
                    oT[:, lo:], v_s[:, kb * HD:(kb + 1) * HD],
                    ptiles[idx][:, lo:],
                    start=(idx == 0), stop=(idx == nkb - 1))
                # interleave remaining denominator matmuls a few slots behind
                # so their DVE pair-sums are ready when PE reaches them
                while ndn < npr - 2 and 2 * ndn + 3 <= idx:
                    nc.tensor.matmul(dn, ones_s, dn_moving[ndn],
                                     start=(ndn == 0), stop=(ndn == npr - 1))
                    ndn += 1
            while ndn < npr:
                nc.tensor.matmul(dn, ones_s, dn_moving[ndn],
                                 start=(ndn == 0), stop=(ndn == npr - 1))
                ndn += 1
            # reciprocal + normalize in halves so the first wo matmuls of the
            # quarter can start before the full 512-wide normalize finishes
            rec = rcp.tile([128, 512], f32, tag="rec")
            aTh = aT_s[h][:, qc * 512:(qc + 1) * 512]
            for c in (0, 256):
                nc.vector.reciprocal_approx_fast(
                    rec[:, c:c + 256], dn[:, c:c + 256])
                nc.vector.tensor_mul(
                    aTh[:, c:c + 256], oT[:, c:c + 256], rec[:, c:c + 256])

        def wo_block(qc):
            for tb in range(qc * 4, qc * 4 + 4):
                ob = ostage.tile([128, DIM], bf16, tag="ob")
                for n4 in range(4):
                    op = wops.tile([128, 512], f32, tag="op")
                    for h in range(HPC):
                        nc.tensor.matmul(
                            op, aT_s[h][:, tb * 128:(tb + 1) * 128],
                            wo_s[:, h, n4 * 512:(n4 + 1) * 512],
                            start=(h == 0), stop=(h == HPC - 1))
                    if n4 % 2 == 0:
                        nc.scalar.copy(ob[:, n4 * 512:(n4 + 1) * 512], op)
                    else:
                        nc.vector.tensor_copy(
                            ob[:, n4 * 512:(n4 + 1) * 512], op)
                if qc == NQC - 1:
                    # final quarter: halve each store across both queues so
                    # the last store drains in half the time
                    rows = out[tb * 128:(tb + 1) * 128, :]
                    nc.sync.dma_start(rows[:, 0:DIM // 2], ob[:, 0:DIM // 2])
                    nc.scalar.dma_start(rows[:, DIM // 2:], ob[:, DIM // 2:])
                else:
                    eng = nc.sync if tb % 2 == 0 else nc.scalar
                    eng.dma_start(out[tb * 128:(tb + 1) * 128, :], ob)

        for tq in range(NQC):
            if tq == 0:
                nc.sync.dma_start(x_s[2], xP[256:384, :])
            elif tq == 1:
                nc.sync.dma_start(x_s[3], xP[384:512, :])
            t0 = tq * 512
            xt = x_s[tq]
            # proj + rope, interleaved so PE never waits on ACT copies;
            # h0 scores run between q1-proj and v-proj so the serial ACT
            # exp chain starts ~7us earlier and overlaps the projections
            proj(f"k{tq}", lambda d: wk_s[:, d, :], xt, kT_s, t0)
            proj(f"q0_{tq}", lambda d: wq_s[:, d, 0:HD], xt, qT_s, t0)
            rk = rope_rot(kT_s, t0)
            proj(f"q1_{tq}", lambda d: wq_s[:, d, HD:2 * HD],
                 xt, qT_s, T + t0)
            rope_fin(kT_s, t0, t0, rk)
            rq0 = rope_rot(qT_s, t0)
            rope_fin(qT_s, t0, t0, rq0)
            pt0, pr0 = attn_scores(0, tq)
            proj(f"v{tq}", lambda d: wv_s[:, d, :], xt, vT_stage, t0)
            rq1 = rope_rot(qT_s, T + t0)
            rope_fin(qT_s, T + t0, t0, rq1)
            # early denominator matmuls (ready pairs) cover the v-copy wait
            n0 = max(0, len(pr0) - 4)
            dn0, mv0 = attn_dn(0, tq, pt0, pr0, upto=n0)
            v_transposes(tq)
            # first two h1 score tiles before AV h0: their ACT exps queue
            # right behind the h0 exps instead of idling ACT
            pt1, pr1 = attn_scores(1, tq, kb_lo=0, kb_hi=2)
            attn_av(0, tq, pt0, dn0, mv0, n0)
            attn_scores(1, tq, kb_lo=2, ptiles=pt1, pairs=pr1)
            dn1, mv1 = attn_dn(1, tq, pt1, pr1, upto=0)
            attn_av(1, tq, pt1, dn1, mv1, 0)
            wo_block(tq)

    nc.compile()
    return nc


def _shard_inputs(x, wq, wk, wv, wo, cos, sin):
    import ml_dtypes

    bf = ml_dtypes.bfloat16

    # x packed per t-quarter into SBUF layout: [4*128, 16*512] where row
    # (q*128 + p), col (d*512 + t) = x[q*512 + t, d*128 + p]
    x2 = x.reshape(T, DIM)
    xPh = np.ascontiguousarray(
        x2.reshape(NQC, 512, ND, 128).transpose(0, 3, 2, 1)
        .reshape(NQC * 128, ND * 512)).astype(bf)

    cosTh = np.ascontiguousarray(cos.T).astype(bf)
    # rotate_half sign fold: out = u*cos + u_rot*sin_signed
    sinTh = np.ascontiguousarray(sin.T).copy()
    sinTh[: HD // 2, :] *= -1.0
    sinTh = sinTh.astype(bf)

    def packw(w_slice):
        # [n, DIM] -> [128 p, 16 d, n] -> [128, 16*n]
        n = w_slice.shape[0]
        return np.ascontiguousarray(
            w_slice.reshape(n, ND, 128).transpose(2, 1, 0)
            .reshape(128, ND * n)).astype(bf)

    in_maps = []
    for c in range(NCORES):
        g = c // 2
        in_maps.append({
            "xP": xPh,
            "wqP": packw(wq[c * HPC * HD:(c + 1) * HPC * HD, :]),
            "wkP": packw(wk[g * HD:(g + 1) * HD, :]),
            "wvP": packw(wv[g * HD:(g + 1) * HD, :]),
            "woT": np.ascontiguousarray(
                wo[:, c * HPC * HD:(c + 1) * HPC * HD].T).astype(bf),
            "cosT": cosTh,
            "sinT": sinTh,
        })
    return in_maps


def _get_exec():
    """Build (once) a cached jitted SPMD executable over the 8 cores.

    Mirrors bass2jax.run_bass_via_pjrt's multi-core branch, but caches the
    jitted callable so repeat kernel() calls don't re-trace/re-lower.
    """
    if "exec" in _CACHE:
        return _CACHE["exec"]

    import jax
    from jax.sharding import Mesh, PartitionSpec
    from jax.experimental.shard_map import shard_map
    from concourse import bass2jax
    import concourse.mybir as mybir

    if "nc" not in _CACHE:
        _CACHE["nc"] = _build_nc()
    nc = _CACHE["nc"]

    bass2jax.install_neuronx_cc_hook()

    part_name = (nc.partition_id_tensor.name
                 if nc.partition_id_tensor else None)
    in_names, out_names, out_avals = [], [], []
    for alloc in nc.m.functions[0].allocations:
        if not isinstance(alloc, mybir.MemoryLocationSet):
            continue
        name = alloc.memorylocations[0].name
        if alloc.kind == "ExternalInput":
            if name != part_name:
                in_names.append(name)
        elif alloc.kind == "ExternalOutput":
            out_names.append(name)
            out_avals.append(jax.core.ShapedArray(
                tuple(alloc.tensor_shape), mybir.dt.np(alloc.dtype)))

    bind_names = in_names + out_names
    if part_name is not None:
        bind_names = bind_names + [part_name]

    def _body(*args):
        operands = list(args)
        if part_name is not None:
            operands.append(bass2jax.partition_id_tensor())
        outs = bass2jax._bass_exec_p.bind(
            *operands,
            out_avals=tuple(out_avals),
            in_names=tuple(bind_names),
            out_names=tuple(out_names),
            lowering_input_output_aliases=(),
            sim_require_finite=True,
            sim_require_nnan=True,
            nc=nc,
        )
        return tuple(outs)

    devices = jax.devices()[:NCORES]
    mesh = Mesh(np.asarray(devices), ("core",))
    n_in = len(in_names)
    n_out = len(out_names)
    sharded = jax.jit(
        shard_map(
            _body, mesh=mesh,
            in_specs=(PartitionSpec("core"),) * (n_in + n_out),
            out_specs=(PartitionSpec("core"),) * n_out,
            check_rep=False,
        ),
        donate_argnums=tuple(range(n_in, n_in + n_out)),
        keep_unused=True,
    )
    _CACHE["body"] = _body
    _CACHE["exec"] = (sharded, in_names, out_names, out_avals, mesh)
    return _CACHE["exec"]


def _concat_inputs(in_maps, in_names):
    return [
        np.concatenate([in_maps[c][name] for c in range(NCORES)], axis=0)
        for name in in_names
    ]


def _zero_outs(out_avals):
    return [
        np.zeros((NCORES * a.shape[0], *a.shape[1:]), a.dtype)
        for a in out_avals
    ]


def kernel(**inputs):
    sharded, in_names, out_names, out_avals, _ = _get_exec()

    in_maps = _shard_inputs(
        np.asarray(inputs["x"], dtype=np.float32),
        np.asarray(inputs["wq"], dtype=np.float32),
        np.asarray(inputs["wk"], dtype=np.float32),
        np.asarray(inputs["wv"], dtype=np.float32),
        np.asarray(inputs["wo"], dtype=np.float32),
        np.asarray(inputs["cos"], dtype=np.float32),
        np.asarray(inputs["sin"], dtype=np.float32),
    )
    concat_in = _concat_inputs(in_maps, in_names)
    out_arrs = sharded(*concat_in, *_zero_outs(out_avals))

    full = np.asarray(out_arrs[out_names.index("out")])
    acc = full.reshape(NCORES, T, DIM).astype(np.float32).sum(axis=0)
    return acc.reshape(1, T, DIM)
